# revision 38
# baseline (speedup 1.0000x reference)
"""DiffPool-like GNN (two GCN convs + softmax clustering + weighted pooling)
as ONE fused Bass/Tile SPMD launch on 8 Trainium2 NeuronCores.

Distribution (matches the sharding hint):
  * nodes partitioned into 8 contiguous shards; each core owns the edges whose
    dst falls in its shard (host buckets edges by 128-node dst window);
  * W1/W2 replicated (const bundle);
  * each core computes h = (D^-1/2 x) @ W rows for ITS OWN shard only, then an
    on-device AllGather assembles the full [Npad, 64] f32 feature table in
    natural node order (the "halo exchange" - here a full gather since edges
    are random);
  * per-edge messages fetched with the MoE dma_gather primitive (256B rows,
    int16 indices -> the table is addressed in 4 parts of Npad/4 rows; padding
    slots point at row 0 and are masked by the one-hot matmul);
  * segment-sum on the tensor engine: per 128-slot chunk, a one-hot matrix
    M[p, r] = (dstloc[p] == r) is built on the vector engine and
    agg += M.T @ msgs accumulates in PSUM across the window's chunks;
  * conv output x1 = dinv*(agg + g_self) + b stays resident in SBUF; the
    conv2 table rows xs1 @ W2 are produced per-window (transpose via an
    identity matmul) and AllGathered the same way;
  * pooling without any gather: per window, onehotB[n, g] = (batch[n] == g)
    over all B=128 graphs and an outer product S[n,k]*x1[n,j] feed
    pooled[g, k*64+j] += onehotB.T @ outer, accumulated in PSUM across all
    windows; a ReduceScatter leaves each core with 16 graph rows ("all-reduce
    the per-(graph,cluster) pooled partial sums");
  * host work: reshape the concatenated ReduceScatter output.

The walrus build in this container encodes at most ONE sync wait per
instruction; _split_waits() rewrites the scheduled BIR, moving excess waits
onto injected single-wait NoOps.
"""

import os
import sys
import numpy as np

sys.path.insert(0, "/opt/trn_rl_repo")

import ml_dtypes  # noqa: E402
import concourse.bacc as bacc  # noqa: E402
import concourse.mybir as mybir  # noqa: E402
import concourse.tile as tile  # noqa: E402
from concourse.tile_rust import add_dep_helper  # noqa: E402

P = 128
BF16 = mybir.dt.bfloat16
F32 = mybir.dt.float32
I16 = mybir.dt.int16
I32 = mybir.dt.int32
NP_BF16 = ml_dtypes.bfloat16

AluOp = mybir.AluOpType
ActFn = mybir.ActivationFunctionType

_DT_MAP = {
    np.dtype(np.float32): F32,
    np.dtype(np.int16): I16,
    np.dtype(np.int8): mybir.dt.int8,
    np.dtype(NP_BF16): BF16,
}

PARTS = 4
GCAP = 8  # chunks per gather instruction (= 64 descs per engine, HW max)


class ConstBundle:
    """Packs [128, n] arrays of mixed dtypes into one [128, W] int32 array."""

    def __init__(self):
        self.fields = {}
        self.nbytes = 0

    def add(self, name, dtype, n):
        dt = np.dtype(dtype)
        b = dt.itemsize * n
        b4 = (b + 3) & ~3
        self.fields[name] = (self.nbytes, dt, n)
        self.nbytes += b4

    def pack(self, arrays):
        w = self.nbytes // 4
        out = np.zeros((P, w), np.int32)
        ob = out.view(np.uint8)
        for name, (off, dt, n) in self.fields.items():
            a = np.ascontiguousarray(arrays[name])
            assert a.dtype == dt and a.shape == (P, n), (name, a.dtype, a.shape)
            ob[:, off:off + dt.itemsize * n] = a.view(np.uint8)
        return out

    def view(self, cb_sb, name):
        off, dt, n = self.fields[name]
        b4 = (dt.itemsize * n + 3) & ~3
        v = cb_sb[:, off // 4:(off + b4) // 4].bitcast(_DT_MAP[dt])
        return v[:, :n]


def _split_waits(nc, budget=1):
    """Move excess sync waits onto injected single-wait same-engine NoOps.
    The walrus in this container encodes at most one wait per instruction."""
    for fn in nc.m.functions:
        for blk in fn.blocks:
            out = []
            for ins in blk.instructions:
                si = ins.sync_info
                if (si is not None and si.on_wait
                        and len(si.on_wait) > budget
                        and ins.opcode not in ("EventSemaphore",)):
                    waits = list(si.on_wait)
                    excess, keep = waits[:-budget], waits[-budget:]
                    for i, wv in enumerate(excess):
                        nop = mybir.InstNoOp(
                            name=f"{ins.name}-sw{i}", engine=ins.engine,
                            bass_nofuse=True,
                            sync_info=mybir.SyncInfo(on_wait=[wv], on_update=[]))
                        out.append(nop)
                    si.on_wait = keep
                out.append(ins)
            blk.instructions[:] = out


def _wrap16_base(flat):
    """dma_gather index layout base: [16, n/16] int16; index j sits at
    [j%16, j//16]. The device replicates it to all 8 groups (128 rows)."""
    n = flat.shape[0]
    assert n % 16 == 0
    return np.ascontiguousarray(flat.reshape(n // 16, 16).T.astype(np.int16))


def _iota_full():
    return np.tile(np.arange(P, dtype=NP_BF16)[None, :], (P, 1))


# =========================================================================
# host-side preprocessing
# =========================================================================

class Meta:
    pass


def preprocess(x_in, edge_index, batch, W1, b1, W2, b2, n_cores=8):
    pr = Meta()
    N, IN = x_in.shape
    D = W1.shape[1]
    K = W2.shape[1]
    assert IN == P

    src = np.ascontiguousarray(edge_index[0]).astype(np.int64)
    dst = np.ascontiguousarray(edge_index[1]).astype(np.int64)
    batch = np.asarray(batch).astype(np.int64)

    WPC = int(np.ceil(N / n_cores / P))
    NS = WPC * P
    Npad = NS * n_cores
    assert Npad % PARTS == 0
    PS = Npad // PARTS
    assert PS < 2 ** 15

    deg = np.bincount(dst, minlength=N).astype(np.float64)
    dinv_pad = np.ones(Npad, np.float32)
    dinv_pad[:N] = (1.0 / np.sqrt(deg + 1.0)).astype(np.float32)

    pr.__dict__.update(dict(
        N=N, B=P, IN=IN, D=D, K=K, n_cores=n_cores, WPC=WPC, NS=NS,
        Npad=Npad, PS=PS,
        W1=W1.astype(np.float32), b1=b1.astype(np.float32),
        W2=W2.astype(np.float32), b2=b2.astype(np.float32),
    ))

    # ---- per-core xs^T shard (xs = x * dinv), bf16 [IN, NS]
    xs = (x_in * dinv_pad[:N, None]).astype(NP_BF16)
    pr.xT = []
    for c in range(n_cores):
        lo, hi = c * NS, min((c + 1) * NS, N)
        blk = np.zeros((IN, NS), NP_BF16)
        blk[:, :hi - lo] = xs[lo:hi].T
        pr.xT.append(blk)

    pr.dinvT = [np.ascontiguousarray(
        dinv_pad[c * NS:(c + 1) * NS].reshape(WPC, P).T)
        for c in range(n_cores)]

    # batch (global graph id 0..127) per shard slot, -1 for pad rows
    bloc_pad = np.full(Npad, -1.0, np.float32)
    bloc_pad[:N] = batch.astype(np.float32)
    pr.batchlocT = [np.ascontiguousarray(
        bloc_pad[c * NS:(c + 1) * NS].reshape(WPC, P).T).astype(NP_BF16)
        for c in range(n_cores)]

    # ---- global edge slotting by (core, dst window, table part, src parity)
    # the f32 h rows are packed two-nodes-per-256B-row in a bf16 table:
    # table row = src // 2 (split into 2 int16-addressable parts), and the
    # src parity selects which 64-col half of the gathered row to use.
    core = dst // NS
    wloc = (dst - core * NS) // P
    pair = src // 2
    part = pair // PS
    cls = src % 2
    key = (((core * WPC + wloc) * 2 + part) * 2 + cls)
    NB = WPC * PARTS  # 4 buckets per window: (part, cls)
    order = np.argsort(key, kind="stable")
    key_o = key[order]
    cnt = np.bincount(key, minlength=n_cores * NB)
    C4 = max(1, int(np.ceil(cnt.max() / P)))
    SL = C4 * P
    NCHUNK = NB * C4
    pr.C4, pr.NCHUNK = C4, NCHUNK

    starts = np.zeros(n_cores * NB + 1, np.int64)
    np.cumsum(cnt, out=starts[1:])
    slots = np.zeros((n_cores * NB, SL), np.int64)
    dloc = np.full((n_cores * NB, SL), -1.0, np.float32)
    pos = np.arange(len(key_o)) - starts[key_o]
    slots[key_o, pos] = pair[order] % PS
    dloc[key_o, pos] = (dst[order] % P).astype(np.float32)

    pr.idx16, pr.dstlocT = [], []
    for c in range(n_cores):
        s = slots[c * NB:(c + 1) * NB]
        d = dloc[c * NB:(c + 1) * NB]
        pr.idx16.append(_wrap16_base(s.reshape(-1)))
        pr.dstlocT.append(np.ascontiguousarray(
            d.reshape(NB, C4, P)
             .transpose(2, 0, 1).reshape(P, NCHUNK)).astype(np.int8))

    # ---- const bundle (layout shared across cores)
    cb = ConstBundle()
    cb.add("dinvT", np.float32, WPC)
    cb.add("bt1", np.float32, D)
    cb.add("bt2", np.float32, K)
    cb.add("w1", NP_BF16, D)
    cb.add("w2", NP_BF16, K)
    cb.add("iota", NP_BF16, P)
    cb.add("piota", NP_BF16, 2)
    cb.add("bloc", NP_BF16, WPC)
    cb.add("dstloc", np.int8, NCHUNK)
    pr.cb = cb
    return pr


def make_in_maps(pr):
    D, K = pr.D, pr.K
    w2pad = np.zeros((P, K), NP_BF16)
    w2pad[:D] = pr.W2.astype(NP_BF16)
    piota = np.zeros((P, 2), NP_BF16)
    piota[:, 0] = np.arange(P, dtype=NP_BF16)
    maps = []
    for c in range(pr.n_cores):
        cb = pr.cb.pack(dict(
            dinvT=pr.dinvT[c],
            bt1=np.tile(pr.b1[None, :], (P, 1)).astype(np.float32),
            bt2=np.tile(pr.b2[None, :], (P, 1)).astype(np.float32),
            w1=pr.W1.astype(NP_BF16), w2=w2pad,
            iota=_iota_full(), piota=piota,
            bloc=pr.batchlocT[c], dstloc=pr.dstlocT[c]))
        maps.append(dict(xT=pr.xT[c], idx=pr.idx16[c], cb=cb))
    return maps


# =========================================================================
# Bass program builder (single fused launch)
# =========================================================================

def _edge_phase(nc, pools, pr, gtab, idxrep_d, dstloc_sb, iota_sb,
                gather_dep, D2, FUSE, finish):
    """Per dst-window: dma_gathers of 2-node 256B rows + M-matmul segment sum.

    gtab is [Npad//2, D2=2*D] bf16; chunk cc's class ((cc // C4) % 2) selects
    which D-col half of each gathered row feeds the matmul."""
    WPC, C4, PS = pr.WPC, pr.C4, pr.PS
    D = D2 // 2
    msp, mtp, pp, ixp = (pools["msgs"], pools["mt"], pools["ps"],
                         pools["ix"])
    WIX = PARTS * C4 * 8  # idx cols per window
    first = [True]
    for w in range(WPC):
        idxw = ixp.tile([P, WIX], I16)
        nc.sync.dma_start(out=idxw[:],
                          in_=idxrep_d[:, w * WIX:(w + 1) * WIX])
        mt = mtp.tile([P, PARTS * C4 * P], BF16)
        nc.vector.tensor_tensor(
            out=mt[:].rearrange("p (k r) -> p k r", r=P),
            in0=dstloc_sb[:, w * PARTS * C4:(w + 1) * PARTS * C4]
                .unsqueeze(2).to_broadcast([P, PARTS * C4, P]),
            in1=iota_sb[:].unsqueeze(1).to_broadcast([P, PARTS * C4, P]),
            op=AluOp.is_equal)
        pst = pp.tile([P, 512], F32, name="pst", tag="seg")
        ps = pst[:, :FUSE]
        NCP = 2 * C4  # chunks per table part (classes 0 and 1)
        for q2 in range(2):
            for k0 in range(0, NCP, GCAP):
                nk = min(GCAP, NCP - k0)
                lc0 = q2 * NCP + k0
                msgs = msp.tile([P, GCAP * D2], BF16)
                g = nc.gpsimd.dma_gather(
                    msgs[:, :nk * D2].rearrange("p (c e) -> p c e", e=D2),
                    gtab[q2 * PS:(q2 + 1) * PS, :],
                    idxw[:, lc0 * 8:(lc0 + nk) * 8],
                    nk * P, nk * P, D2)
                if first[0]:
                    first[0] = False
                    add_dep_helper(g.ins, gather_dep.ins, sync=True,
                                   reason="gather after table allgather")
                for k in range(nk):
                    cc = lc0 + k
                    cls = (cc // C4) % 2
                    nc.tensor.matmul(
                        ps[:],
                        lhsT=mt[:, cc * P:(cc + 1) * P],
                        rhs=msgs[:, k * D2 + cls * D:
                                    k * D2 + cls * D + FUSE],
                        start=(cc == 0),
                        stop=(cc == PARTS * C4 - 1))
        finish(w, ps)


def build(pr, split=True, reps=1):
    from contextlib import ExitStack
    IN, D, K, WPC, Npad, NS, PS = (pr.IN, pr.D, pr.K, pr.WPC, pr.Npad,
                                   pr.NS, pr.PS)
    NCHUNK, C4 = pr.NCHUNK, pr.C4
    CBW = pr.cb.nbytes // 4
    IDXW = NCHUNK * 8
    KD = K * D
    TB = 7
    assert WPC % TB == 0

    nc = bacc.Bacc("TRN2")
    xT_d = nc.declare_dram_parameter("xT", [IN, NS], BF16, isOutput=False)
    idx_d = nc.declare_dram_parameter("idx", [16, IDXW], I16, isOutput=False)
    cb_d = nc.declare_dram_parameter("cb", [P, CBW], I32, isOutput=False)
    pool_d = nc.declare_dram_parameter("pool", [P // 8, KD], F32,
                                       isOutput=True)

    gtab1 = nc.dram_tensor("gtab1", [Npad // 2, 2 * D], BF16)
    gtab2 = nc.dram_tensor("gtab2", [Npad // 2, 2 * D], BF16)
    idxrep_d = nc.dram_tensor("idxrep", [P, IDXW], I16)
    ag1_in = nc.dram_tensor("ag1in", [NS, D], BF16)
    ag2_in = nc.dram_tensor("ag2in", [NS, D], BF16)
    rs_in = nc.dram_tensor("rsin", [P, KD], F32)
    rs_out = nc.dram_tensor("rsout", [P // 8, KD], F32)

    with tile.TileContext(nc) as tc, ExitStack() as es:
        pools = {}
        for nm, bufs, space in [
                ("const", 1, None), ("msgs", 6, None),
                ("mt", 3, None), ("xw", 4, None), ("hw", 3, None),
                ("ix", 3, None),
                ("ps", 2, "PSUM"), ("psb", 2, "PSUM")]:
            kw = dict(name=nm, bufs=bufs)
            if space:
                kw["space"] = space
            pools[nm] = es.enter_context(tc.tile_pool(**kw))
        cp = pools["const"]

        cb_sb = cp.tile([P, CBW], I32, name="cb_sb", tag="cb_sb")
        nc.sync.dma_start(out=cb_sb[:], in_=cb_d[:])
        nc.vector.tensor_copy(out=cb_sb[:], in_=cb_sb[:])
        V = lambda name: pr.cb.view(cb_sb, name)
        dinvT_sb, iota_sb = V("dinvT"), V("iota")

        # replicate the 16-row idx base to all 128 partition rows, in DRAM;
        # per-window slices stream back into small SBUF tiles in _edge_phase
        for g in range(8):
            nc.sync.dma_start(out=idxrep_d[g * 16:(g + 1) * 16, :],
                              in_=idx_d[:])

        dstloc_sb = cp.tile([P, NCHUNK], BF16, name="dstloc_sb",
                            tag="dstloc_sb")
        nc.vector.tensor_copy(out=dstloc_sb[:], in_=V("dstloc"))

        g1_shard = cp.tile([P, WPC * D], F32)
        g2_shard = cp.tile([P, WPC * K], F32)
        x1_sb = cp.tile([P, WPC * D], BF16)
        ident = cp.tile([P, P], BF16, name="ident", tag="ident")
        nc.vector.tensor_tensor(
            out=ident[:],
            in0=V("piota")[:, :1].to_broadcast([P, P]),
            in1=iota_sb[:], op=AluOp.is_equal)

        bt1_sb, w2_sb = V("bt1"), V("w2")
        bt2_sb, bloc_sb = V("bt2"), V("bloc")
        xwp, hwp = pools["xw"], pools["hw"]
        plq_pool = es.enter_context(tc.tile_pool(name="plq", bufs=1,
                                                 space="PSUM"))
        NQ = KD // 512
        state = {}

        def phase1(r):
            # ---- own-shard h1 rows -> ag1_in; AllGather -> gtab1
            with tc.tile_pool(name=f"xt{r}", bufs=1) as xtp:
                xt = xtp.tile([IN, NS], BF16, name="xt", tag="xt")
                nc.sync.dma_start(out=xt[:], in_=xT_d[:])
                w1_sb = V("w1")
                writes = []
                for gi in range(WPC // TB):
                    ps8 = pools["ps"].tile([P, 512], F32, name="ps8",
                                           tag="seg")
                    for j in range(TB):
                        t = gi * TB + j
                        nc.tensor.matmul(ps8[:, j * D:(j + 1) * D],
                                         lhsT=xt[:, t * P:(t + 1) * P],
                                         rhs=w1_sb[:], start=True, stop=True)
                    nc.vector.tensor_copy(
                        out=g1_shard[:, gi * TB * D:(gi + 1) * TB * D],
                        in_=ps8[:, :TB * D])
                    g8 = hwp.tile([P, TB * D], BF16, name="g8", tag="g8")
                    nc.scalar.copy(out=g8[:], in_=ps8[:, :TB * D])
                    w = nc.sync.dma_start(
                        out=ag1_in[gi * TB * P:(gi + 1) * TB * P, :]
                            .rearrange("(t p) d -> p t d", p=P),
                        in_=g8[:].rearrange("p (t d) -> p t d", d=D))
                    writes.append(w)
            ag1 = nc.gpsimd.collective_compute(
                "AllGather", AluOp.bypass,
                replica_groups=[list(range(pr.n_cores))],
                ins=[ag1_in[:, :]], outs=[gtab1[:, :]])
            for w in writes:
                add_dep_helper(ag1.ins, w.ins, sync=True, reason="ag1 after h1")
            return ag1

        def finish1(w, ps):
            h2_writes = state["h2_writes"]
            t1 = xwp.tile([P, D], F32, tag="t1")
            nc.vector.tensor_tensor(out=t1[:], in0=ps[:],
                                    in1=g1_shard[:, w * D:(w + 1) * D],
                                    op=AluOp.add)
            xf = xwp.tile([P, D], F32, tag="xf")
            nc.vector.tensor_scalar(
                out=xf[:], in0=t1[:], scalar1=dinvT_sb[:, w:w + 1],
                scalar2=None, op0=AluOp.mult)
            nc.vector.tensor_tensor(out=x1_sb[:, w * D:(w + 1) * D],
                                    in0=xf[:], in1=bt1_sb[:], op=AluOp.add)
            xsq = xwp.tile([P, D], BF16, tag="xsq")
            nc.vector.tensor_scalar(
                out=xsq[:], in0=x1_sb[:, w * D:(w + 1) * D],
                scalar1=dinvT_sb[:, w:w + 1], scalar2=None, op0=AluOp.mult)
            # transpose xs1 via identity matmul, then h2 = xs1 @ W2
            pstt = pools["psb"].tile([P, 512], F32, name="pstt", tag="psb")
            pst = pstt[:D, :P]
            nc.tensor.matmul(pst, lhsT=xsq[:], rhs=ident[:],
                             start=True, stop=True)
            xst = xwp.tile([D, P], BF16, tag="xst")
            nc.scalar.copy(out=xst[:], in_=pst)
            ph2t = pools["psb"].tile([P, 512], F32, name="ph2t", tag="psb")
            ph2 = ph2t[:, :K]
            nc.tensor.matmul(ph2, lhsT=xst[:], rhs=w2_sb[:D, :],
                             start=True, stop=True)
            nc.vector.tensor_copy(out=g2_shard[:, w * K:(w + 1) * K],
                                  in_=ph2)
            h8 = hwp.tile([P, D], BF16)
            nc.scalar.copy(out=h8[:, :K], in_=ph2)
            nc.scalar.activation(out=h8[:, K:], in_=ph2,
                                 func=ActFn.Copy, scale=0.0)
            hw_ = nc.sync.dma_start(out=ag2_in[w * P:(w + 1) * P, :],
                                    in_=h8[:])
            h2_writes.append(hw_)

        def phase2(r, ag1):
            # ---- conv1 edge phase
            state["h2_writes"] = []
            _edge_phase(nc, pools, pr, gtab1, idxrep_d, dstloc_sb, iota_sb,
                        ag1, 2 * D, D, finish1)
            ag2 = nc.gpsimd.collective_compute(
                "AllGather", AluOp.bypass,
                replica_groups=[list(range(pr.n_cores))],
                ins=[ag2_in[:, :]], outs=[gtab2[:, :]])
            for w in state["h2_writes"]:
                add_dep_helper(ag2.ins, w.ins, sync=True, reason="ag2 after h2")
            return ag2

        def finish2(w, ps):
            plq = state["plq"]
            t1 = xwp.tile([P, K], F32, tag="t1")
            nc.vector.tensor_tensor(out=t1[:], in0=ps[:],
                                    in1=g2_shard[:, w * K:(w + 1) * K],
                                    op=AluOp.add)
            sl = xwp.tile([P, K], F32, tag="xf")
            nc.vector.tensor_scalar(
                out=sl[:], in0=t1[:], scalar1=dinvT_sb[:, w:w + 1],
                scalar2=None, op0=AluOp.mult)
            sl2 = xwp.tile([P, K], F32, tag="sl2")
            nc.vector.tensor_tensor(out=sl2[:], in0=sl[:], in1=bt2_sb[:],
                                    op=AluOp.add)
            ex = xwp.tile([P, K], F32, tag="ex")
            nc.scalar.activation(out=ex[:], in_=sl2[:], func=ActFn.Exp)
            sm = xwp.tile([P, 1], F32, tag="sm")
            nc.vector.tensor_reduce(out=sm[:], in_=ex[:],
                                    axis=mybir.AxisListType.X, op=AluOp.add)
            rc = xwp.tile([P, 1], F32, tag="rc")
            nc.vector.reciprocal(out=rc[:], in_=sm[:])
            sq = xwp.tile([P, K], BF16, tag="sq")
            nc.vector.tensor_scalar(
                out=sq[:], in0=ex[:], scalar1=rc[:, :1], scalar2=None,
                op0=AluOp.mult)
            ob = xwp.tile([P, P], BF16, tag="ob")
            nc.vector.tensor_tensor(
                out=ob[:], in0=bloc_sb[:, w:w + 1].to_broadcast([P, P]),
                in1=iota_sb[:], op=AluOp.is_equal)
            outer = hwp.tile([P, KD], BF16)
            nc.vector.tensor_tensor(
                out=outer[:].rearrange("p (k j) -> p k j", j=D),
                in0=sq[:].unsqueeze(2).to_broadcast([P, K, D]),
                in1=x1_sb[:, w * D:(w + 1) * D]
                    .unsqueeze(1).to_broadcast([P, K, D]),
                op=AluOp.mult)
            for i in range(NQ):
                nc.tensor.matmul(plq[i][:], lhsT=ob[:],
                                 rhs=outer[:, i * 512:(i + 1) * 512],
                                 start=(w == 0), stop=(w == WPC - 1))

        def phase3(r, ag2):
            # ---- conv2 edge phase + pooling accumulation + ReduceScatter
            state["plq"] = [
                plq_pool.tile([P, 512], F32, name=f"plq{i}", tag=f"plq{i}")
                for i in range(NQ)]
            _edge_phase(nc, pools, pr, gtab2, idxrep_d, dstloc_sb, iota_sb,
                        ag2, 2 * D, K, finish2)
            pool_sb = cp.tile([P, KD], F32, name="pool_sb", tag="pool_sb")
            for i in range(NQ):
                nc.vector.tensor_copy(out=pool_sb[:, i * 512:(i + 1) * 512],
                                      in_=state["plq"][i][:])
            rw = nc.sync.dma_start(out=rs_in[:, :], in_=pool_sb[:])
            rs = nc.gpsimd.collective_compute(
                "ReduceScatter", AluOp.add,
                replica_groups=[list(range(pr.n_cores))],
                ins=[rs_in[:, :]], outs=[rs_out[:, :]])
            add_dep_helper(rs.ins, rw.ins, sync=True, reason="rs after pool")
            return rs

        rs = None
        for r in range(reps):
            ag1 = phase1(r)
            if rs is not None:  # serialize reps (slope microbenchmark)
                add_dep_helper(ag1.ins, rs.ins, sync=True, reason="rep chain")
            ag2 = phase2(r, ag1)
            rs = phase3(r, ag2)

        out_sb = cp.tile([P // 8, KD], F32, name="out_sb", tag="out_sb")
        rd = nc.sync.dma_start(out=out_sb[:], in_=rs_out[:, :])
        add_dep_helper(rd.ins, rs.ins, sync=True, reason="read after rs")
        nc.sync.dma_start(out=pool_d[:], in_=out_sb[:])
    nc.compile()
    if split:
        _split_waits(nc)
    return nc


# =========================================================================
# runner + glue
# =========================================================================

_EXEC_CACHE = {}


def exec_spmd(nc, in_maps):
    """Execute a prebuilt Bass module on len(in_maps) cores via PJRT.

    Mirrors concourse.bass2jax.run_bass_via_pjrt, but (a) caches the jitted
    callable per-module so repeated runs don't re-trace/re-compile XLA, and
    (b) fetches each output as ONE global [n_cores*rows, cols] array (one
    device round-trip) instead of per-core sliced fetches.
    Returns {name: global np.ndarray} with per-core rows concatenated.
    """
    import jax
    from jax.sharding import Mesh, PartitionSpec
    from jax.experimental.shard_map import shard_map
    from concourse import bass2jax, mybir as _mybir
    from concourse.bass2jax import (_bass_exec_p, install_neuronx_cc_hook,
                                    partition_id_tensor)

    n_cores = len(in_maps)
    key = id(nc)
    if key not in _EXEC_CACHE:
        install_neuronx_cc_hook()
        assert nc.dbg_addr is None or not nc.dbg_callbacks
        partition_name = (nc.partition_id_tensor.name
                          if nc.partition_id_tensor else None)
        in_names, out_names, out_avals, zero_outs = [], [], [], []
        for alloc in nc.m.functions[0].allocations:
            if not isinstance(alloc, _mybir.MemoryLocationSet):
                continue
            name = alloc.memorylocations[0].name
            if alloc.kind == "ExternalInput":
                if name != partition_name:
                    in_names.append(name)
            elif alloc.kind == "ExternalOutput":
                shape = tuple(alloc.tensor_shape)
                dtype = _mybir.dt.np(alloc.dtype)
                out_names.append(name)
                out_avals.append(jax.core.ShapedArray(shape, dtype))
                zero_outs.append(np.zeros(shape, dtype))
        n_params = len(in_names)
        all_in = list(in_names) + list(out_names)
        if partition_name is not None:
            all_in.append(partition_name)
        donate = tuple(range(n_params, n_params + len(out_avals)))

        def _body(*args):
            operands = list(args)
            if partition_name is not None:
                operands.append(partition_id_tensor())
            return tuple(_bass_exec_p.bind(
                *operands, out_avals=tuple(out_avals), in_names=tuple(all_in),
                out_names=tuple(out_names), lowering_input_output_aliases=(),
                sim_require_finite=True, sim_require_nnan=True, nc=nc))

        mesh = Mesh(np.asarray(jax.devices()[:n_cores]), ("core",))
        specs = (PartitionSpec("core"),) * (n_params + len(out_avals))
        fn = jax.jit(
            shard_map(_body, mesh=mesh, in_specs=specs,
                      out_specs=(PartitionSpec("core"),) * len(out_names),
                      check_rep=False),
            donate_argnums=donate, keep_unused=True)
        _EXEC_CACHE[key] = (fn, in_names, out_names, zero_outs)

    fn, in_names, out_names, zero_outs = _EXEC_CACHE[key]
    concat_in = [np.concatenate([np.asarray(m[nm]) for m in in_maps], axis=0)
                 for nm in in_names]
    concat_zeros = [np.zeros((n_cores * z.shape[0], *z.shape[1:]), z.dtype)
                    for z in zero_outs]
    out_arrs = fn(*concat_in, *concat_zeros)
    return {nm: np.asarray(a) for nm, a in zip(out_names, out_arrs)}


def kernel(x_in, edge_index, batch, W1, b1, W2, b2):
    n_cores = 8
    pr = preprocess(x_in, edge_index, batch, W1, b1, W2, b2, n_cores)
    nc = build(pr)
    out = exec_spmd(nc, make_in_maps(pr))
    return np.ascontiguousarray(
        out["pool"].reshape(pr.B, pr.K, pr.D).astype(np.float32))
